# revision 1
# baseline (speedup 1.0000x reference)
"""Trainium2 Bass kernel for nn_Conv2D_80796924772741.

Depthwise (grouped, F=64) 3x3 valid conv over [F, 514, 514, 4] int8 with
per-channel int8 weights + int32 bias, followed by exact fixed-point requant
  res = (acc * 19920 + 2^21) >> 22 ;  out = clip(res - 5, -128, 127) int8
(reduced_mantissa 19920 = 1245 * 16 -> res = (acc*1245 + 2^17) >> 18).

Sharding: F=64 split across 8 NeuronCores (8 channels each), embarrassingly
parallel.

Per-core compute, per (channel, H-window) group ([M<=124 rows, 2048 cols];
the last 16 output rows are packed 4-chunks-into-partitions as a [64, 512]
"strip" group so they cost 512-wide ops instead of 4 full-width passes):
 - PE: conv via Toeplitz-band stationary matmuls over H-windows
   (contraction = input rows; all 3 H-taps in the band diagonals; one
   matmul per 512-col chunk per W-tap, W-shift = +4n free-dim offset).
   Bias b rides two all-ones rhs partitions. PSUM = acc+b exact (fp32).
 - ACT1: hi16 = int16(ps * 2^-7 - 0.498046875)  [RNE+sat int conversion
   == floor((acc+b)/128); value grid 1/128 -> no ties]
 - DVE:  hif  = fp16(hi16)    [exact, |hi| <= 1168]
 - PE:   ps  += (-128*I) @ hif  -> lo = (acc+b) mod 128 in PSUM
 - ACT2: q16  = int16(ps * 9.7265625 - 0.498046875)  [= floor(lo*1245/128)]
 - DVE:  S32  = hi16 * 1245 + q16   (scalar_tensor_tensor; exact < 2^21)
 - DVE:  out  = int8(S32 * 2^-11 - 4.999755859375)
   [RNE -> floor(S/2^11 + 1/2) - 5 = res - 5; int8 SATURATION == clip]
All intermediates exact in fp32; conversion semantics (RNE+saturate on both
ACT and DVE, fp32-internal ALU) verified on hardware. Bit-exact vs the
int64 reference.
"""

import numpy as np
import ml_dtypes

F_PER_CORE = 8
H_IN = 514
W_IN = 514
D = 4
H_OUT = 512
WD_OUT = 2048  # 512 * 4
FREE_IN = W_IN * D  # 2056
N_CHUNK = 512
N_CORES = 8

# Full H windows (M=124); rows 496..511 are handled by the packed strip.
FULL_WINDOWS = [(0, 124), (124, 124), (248, 124), (372, 124)]
STRIP_R0 = 496
STRIP_M = 16  # output rows per chunk block
STRIP_KB = 20  # partitions per chunk block: 2 ones + 16+2 data rows


def _build_lhsT(w_core: np.ndarray, b_core: np.ndarray) -> np.ndarray:
    """[128, 8*3*124] bf16 stationary: per (channel, w-tap) a Toeplitz band.

    Layout column block (f*3 + n)*124 : +124  holds T_n for channel f.
    T_n[2 + i + m, i] = w[f, m, n]  (rows 2.. are conv data partitions)
    T_0[0, i] = 8*floor(b/8) ; T_0[1, i] = b mod 8  (bias rows, multiplied
    by all-ones rhs partitions 0/1; both parts bf16-exact).
    """
    out = np.zeros((128, F_PER_CORE * 3 * 124), dtype=np.float32)
    for f in range(F_PER_CORE):
        b_f = int(b_core[f])
        bh = b_f >> 3  # floor division
        bl = b_f - 8 * bh
        for n in range(3):
            base = (f * 3 + n) * 124
            if n == 0:
                out[0, base : base + 124] = float(8 * bh)
                out[1, base : base + 124] = float(bl)
            for m in range(3):
                wv = float(int(w_core[f, m, n, 0]))
                idx = np.arange(124)
                out[2 + idx + m, base + idx] = wv
    return out.astype(ml_dtypes.bfloat16)


def _build_lhsT2(w_core: np.ndarray, b_core: np.ndarray) -> np.ndarray:
    """[80, 8*3*64] bf16 strip stationaries, block-diagonal per chunk.

    Chunk block c occupies partitions 20c..20c+19 (2 ones rows + 18 data
    rows) and psum rows 16c..16c+15. Column block (f*3+n)*64 holds the
    tap-n stationary for channel f covering all 4 chunks.
    """
    out = np.zeros((80, F_PER_CORE * 3 * 64), dtype=np.float32)
    for f in range(F_PER_CORE):
        b_f = int(b_core[f])
        bh = b_f >> 3
        bl = b_f - 8 * bh
        for n in range(3):
            base = (f * 3 + n) * 64
            for c in range(4):
                col0 = base + 16 * c
                row0 = 20 * c
                if n == 0:
                    out[row0 + 0, col0 : col0 + 16] = float(8 * bh)
                    out[row0 + 1, col0 : col0 + 16] = float(bl)
                for m in range(3):
                    wv = float(int(w_core[f, m, n, 0]))
                    idx = np.arange(16)
                    out[row0 + 2 + idx + m, col0 + idx] = wv
    return out.astype(ml_dtypes.bfloat16)


_PROGRAM_CACHE = {}


def _build_program():
    import concourse.bass as bass
    import concourse.tile as tile
    from concourse import bacc, mybir

    nc = bacc.Bacc(
        "TRN2", target_bir_lowering=False, debug=False, num_devices=N_CORES
    )
    dt = mybir.dt
    Alu = mybir.AluOpType
    Act = mybir.ActivationFunctionType

    x_d = nc.dram_tensor(
        "x", [F_PER_CORE, H_IN, FREE_IN], dt.int8, kind="ExternalInput"
    ).ap()
    lhsT_d = nc.dram_tensor(
        "lhsT", [128, F_PER_CORE * 3 * 124], dt.bfloat16, kind="ExternalInput"
    ).ap()
    lhsT2_d = nc.dram_tensor(
        "lhsT2", [80, F_PER_CORE * 3 * 64], dt.bfloat16, kind="ExternalInput"
    ).ap()
    id_d = nc.dram_tensor("id4", [124, 124], dt.float16, kind="ExternalInput").ap()
    ones_d = nc.dram_tensor("ones2", [2, FREE_IN], dt.bfloat16, kind="ExternalInput").ap()
    y_d = nc.dram_tensor(
        "y", [F_PER_CORE, H_OUT, WD_OUT], dt.int8, kind="ExternalOutput"
    ).ap()

    groups = []
    for f in range(F_PER_CORE):
        for (r0, m_r) in FULL_WINDOWS:
            groups.append(("full", f, r0, m_r))
        groups.append(("strip", f, STRIP_R0, STRIP_M))

    with tile.TileContext(nc) as tc:
        with (
            tc.tile_pool(name="const", bufs=1) as const_pool,
            tc.tile_pool(name="xin", bufs=3) as x_pool,
            tc.tile_pool(name="psum", bufs=2, space="PSUM") as psum_pool,
            tc.tile_pool(name="hi16", bufs=2) as hi_pool,
            tc.tile_pool(name="hif", bufs=2) as hif_pool,
            tc.tile_pool(name="q16", bufs=2) as q_pool,
            tc.tile_pool(name="s32", bufs=2) as s_pool,
            tc.tile_pool(name="out8", bufs=3) as o_pool,
        ):
            lhsT_t = const_pool.tile([128, F_PER_CORE * 3 * 124], dt.bfloat16)
            nc.sync.dma_start(lhsT_t[:], lhsT_d[:])
            lhsT2_t = const_pool.tile([80, F_PER_CORE * 3 * 64], dt.bfloat16)
            nc.sync.dma_start(lhsT2_t[:], lhsT2_d[:])
            id_t = const_pool.tile([124, 124], dt.float16)
            nc.sync.dma_start(id_t[:], id_d[:])

            def fixup(prev):
                kind, f, r0, m_r, ps, hi16, hif = prev
                if kind == "full":
                    for c in range(4):
                        nc.tensor.matmul(
                            ps[0:m_r, c * N_CHUNK : (c + 1) * N_CHUNK],
                            id_t[0:m_r, 0:m_r],
                            hif[0:m_r, c * N_CHUNK : (c + 1) * N_CHUNK],
                            start=False,
                            stop=True,
                            skip_group_check=True,
                        )
                else:
                    nc.tensor.matmul(
                        ps[0:64, 0:N_CHUNK],
                        id_t[0:64, 0:64],
                        hif[0:64, 0:N_CHUNK],
                        start=False,
                        stop=True,
                        skip_group_check=True,
                    )

            def phase2(prev):
                kind, f, r0, m_r, ps, hi16, hif = prev
                rows = m_r if kind == "full" else 64
                fd = WD_OUT if kind == "full" else N_CHUNK
                # q16 = floor(lo * 1245/128)
                q16 = q_pool.tile([124, WD_OUT], dt.int16)
                nc.scalar.activation(
                    q16[0:rows, 0:fd], ps[0:rows, 0:fd], Act.Copy,
                    bias=-0.498046875, scale=9.7265625,
                )
                # S = hi*1245 + q  (exact, < 2^21)
                s32 = s_pool.tile([124, WD_OUT], dt.int32)
                nc.vector.scalar_tensor_tensor(
                    s32[0:rows, 0:fd], hi16[0:rows, 0:fd], 1245.0,
                    q16[0:rows, 0:fd], Alu.mult, Alu.add,
                )
                # out = sat8(RNE(S*2^-11 - 4.999755859375)) = clip(res-5)
                o8 = o_pool.tile([124, WD_OUT], dt.int8)
                nc.vector.tensor_scalar(
                    o8[0:rows, 0:fd], s32[0:rows, 0:fd], 0.00048828125,
                    -4.999755859375, Alu.mult, Alu.add,
                )
                if kind == "full":
                    nc.sync.dma_start(y_d[f, r0 : r0 + m_r, :], o8[0:m_r, :])
                else:
                    for c in range(4):
                        nc.sync.dma_start(
                            y_d[f, r0 : r0 + STRIP_M,
                                c * N_CHUNK : (c + 1) * N_CHUNK],
                            o8[16 * c : 16 * c + 16, 0:N_CHUNK],
                        )

            prev = None
            for (kind, f, r0, m_r) in groups:
                if kind == "full":
                    k_r = m_r + 4  # 2 ones rows + m_r + 2 data rows
                    xt = x_pool.tile([128, FREE_IN], dt.bfloat16)
                    nc.sync.dma_start(xt[0:2, :], ones_d[:])
                    nc.gpsimd.dma_start(
                        xt[2 : 2 + m_r + 2, :], x_d[f, r0 : r0 + m_r + 2, :]
                    )
                    ps = psum_pool.tile([124, WD_OUT], dt.float32)
                    for n in range(3):
                        base = (f * 3 + n) * 124
                        for c in range(4):
                            nc.tensor.matmul(
                                ps[0:m_r, c * N_CHUNK : (c + 1) * N_CHUNK],
                                lhsT_t[0:k_r, base : base + m_r],
                                xt[0:k_r, c * N_CHUNK + 4 * n : c * N_CHUNK + 4 * n + N_CHUNK],
                                start=(n == 0),
                                stop=False,
                                skip_group_check=True,
                            )
                    rows, fd = m_r, WD_OUT
                else:
                    xt = x_pool.tile([128, FREE_IN], dt.bfloat16)
                    for c in range(4):
                        p0 = STRIP_KB * c
                        nc.sync.dma_start(xt[p0 : p0 + 2, 0:520], ones_d[:, 0:520])
                        nc.gpsimd.dma_start(
                            xt[p0 + 2 : p0 + STRIP_KB, 0:520],
                            x_d[f, r0 : r0 + 18, c * N_CHUNK : c * N_CHUNK + 520],
                        )
                    ps = psum_pool.tile([124, WD_OUT], dt.float32)
                    for n in range(3):
                        base = (f * 3 + n) * 64
                        nc.tensor.matmul(
                            ps[0:64, 0:N_CHUNK],
                            lhsT2_t[0:80, base : base + 64],
                            xt[0:80, 4 * n : 4 * n + N_CHUNK],
                            start=(n == 0),
                            stop=False,
                            skip_group_check=True,
                        )
                    rows, fd = 64, N_CHUNK

                if prev is not None:
                    fixup(prev)
                    phase2(prev)
                # hi16 = floor((acc+b)/128)  via RNE(x - 0.498046875)
                hi16 = hi_pool.tile([124, WD_OUT], dt.int16)
                nc.scalar.activation(
                    hi16[0:rows, 0:fd], ps[0:rows, 0:fd], Act.Copy,
                    bias=-0.498046875, scale=0.0078125,
                )
                hif = hif_pool.tile([124, WD_OUT], dt.float16)
                nc.vector.tensor_scalar(
                    hif[0:rows, 0:fd], hi16[0:rows, 0:fd], 0, None, Alu.add
                )
                prev = (kind, f, r0, m_r, ps, hi16, hif)
            fixup(prev)
            phase2(prev)

    nc.compile()
    return nc


def _make_in_maps(x: np.ndarray, w: np.ndarray, b: np.ndarray) -> list:
    id4 = (-128.0 * np.eye(124, dtype=np.float32)).astype(np.float16)
    ones2 = np.ones((2, FREE_IN), dtype=np.float32).astype(ml_dtypes.bfloat16)
    in_maps = []
    for core in range(N_CORES):
        lo = core * F_PER_CORE
        hi = lo + F_PER_CORE
        x_shard = np.ascontiguousarray(x[lo:hi]).reshape(F_PER_CORE, H_IN, FREE_IN)
        lhsT = _build_lhsT(w[lo:hi], b[lo:hi])
        lhsT2 = _build_lhsT2(w[lo:hi], b[lo:hi])
        in_maps.append(
            {"x": x_shard, "lhsT": lhsT, "lhsT2": lhsT2, "id4": id4, "ones2": ones2}
        )
    return in_maps


def kernel(x: np.ndarray, w: np.ndarray, b: np.ndarray) -> np.ndarray:
    """x: int8 [64, 514, 514, 4]; w: int8 [64, 3, 3, 1]; b: int32 [64].

    Returns int8 [64, 512, 512, 4].
    """
    from concourse.bass_utils import run_bass_kernel_spmd

    if "nc" not in _PROGRAM_CACHE:
        _PROGRAM_CACHE["nc"] = _build_program()
    nc = _PROGRAM_CACHE["nc"]

    F = x.shape[0]
    assert F == N_CORES * F_PER_CORE

    in_maps = _make_in_maps(x, w, b)
    res = run_bass_kernel_spmd(nc, in_maps, core_ids=list(range(N_CORES)))

    out = np.empty((F, H_OUT, 512, D), dtype=np.int8)
    for core in range(N_CORES):
        lo = core * F_PER_CORE
        y = res.results[core]["y"]  # [8, 512, 2048] int8
        out[lo : lo + F_PER_CORE] = y.reshape(F_PER_CORE, H_OUT, 512, D)
    return out



# revision 2
# speedup vs baseline: 1.2261x; 1.2261x over previous
"""Trainium2 Bass kernel for nn_Conv2D_80796924772741.

Depthwise (grouped, F=64) 3x3 valid conv over [F, 514, 514, 4] int8 with
per-channel int8 weights + int32 bias, followed by exact fixed-point requant
  res = (acc * 19920 + 2^21) >> 22 ;  out = clip(res - 5, -128, 127) int8
(reduced_mantissa 19920 = 1245 * 16 -> res = (acc*1245 + 2^17) >> 18).

Sharding: F=64 split across 8 NeuronCores (8 channels each).

Per-core pipeline, per (channel, H-window) group ([M<=124 rows, 2048 cols];
last 16 output rows packed 4-chunks-into-partitions as a [64, 512] strip):
 - PE:  conv via Toeplitz-band stationary matmuls over H-windows (3 W-taps,
        H-taps in the band diagonals, bias rides two all-ones rhs rows).
        PSUM = A = acc + b, exact fp32 (|A| <= 146161 < 2^24).
 - ACT: af32 = Copy(PSUM) -> SBUF fp32 (exact).
 - requant, one of two exact lanes (both verified bit-exact on HW for every
   possible A):
   lane G (GpSimd, 1 op, internally > fp32 precision):
        o8 = sat8(RNE(af * 1245/2^18 + (2^-19 - 5)))  == clip(res - 5)
   lane D (DVE, 6 ops, split A = 128*(h'+8) + (lo+1024)):
        hi16 = int16(af*2^-7 - 8.498046875)           = floor(A/128) - 8
        lo16 = int16(hi16*-128 + af)                  = A - 128*hi' (in [1024,1152))
        q16  = int16(lo16*9.7265625 - 9960.49609375)  = floor(lo*1245/128)
        qt16 = fp16(q16*2^-11 - 0.136474609375)       (exact in fp16)
        S16  = int16(hi16*0.60791015625 + qt16)       = res - 5 (RNE, no ties)
        o8   = int8(S16)                              (saturating == clip)
 - DMA y (scalar-engine HWDGE queue, delayed 2 groups to avoid HOL).
x is fed pre-converted to bf16 by the host with the two all-ones bias rows
baked in front of each window, so every x load is a single contiguous DMA.
"""

import numpy as np
import ml_dtypes

F_PER_CORE = 8
H_IN = 514
W_IN = 514
D = 4
H_OUT = 512
WD_OUT = 2048  # 512 * 4
FREE_IN = W_IN * D  # 2056
N_CHUNK = 512
N_CORES = 8

FULL_WINDOWS = [(0, 124), (124, 124), (248, 124), (372, 124)]
STRIP_R0 = 496
STRIP_M = 16  # output rows per chunk block
STRIP_KB = 20  # partitions per chunk block: 2 ones + 16+2 data rows

QS = 0.004749298095703125  # 1245 / 2^18
QD = (2.0 ** -19) - 5.0

DVE_LANE_FRAC = 3  # lane D on full groups with (idx % 10) < DVE_LANE_FRAC


def _build_lhsT(w_core: np.ndarray, b_core: np.ndarray) -> np.ndarray:
    """[128, 8*3*124] bf16 stationary: per (channel, w-tap) a Toeplitz band.

    Column block (f*3 + n)*124 : +124 holds T_n for channel f.
    T_n[2 + i + m, i] = w[f, m, n]  (rows 2.. are conv data partitions)
    T_0[0, i] = 8*floor(b/8) ; T_0[1, i] = b mod 8  (bias rows, multiplied
    by all-ones rhs partitions 0/1; both parts bf16-exact).
    """
    out = np.zeros((128, F_PER_CORE * 3 * 124), dtype=np.float32)
    for f in range(F_PER_CORE):
        b_f = int(b_core[f])
        bh = b_f >> 3
        bl = b_f - 8 * bh
        for n in range(3):
            base = (f * 3 + n) * 124
            if n == 0:
                out[0, base : base + 124] = float(8 * bh)
                out[1, base : base + 124] = float(bl)
            for m in range(3):
                wv = float(int(w_core[f, m, n, 0]))
                idx = np.arange(124)
                out[2 + idx + m, base + idx] = wv
    return out.astype(ml_dtypes.bfloat16)


def _build_lhsT2(w_core: np.ndarray, b_core: np.ndarray) -> np.ndarray:
    """[80, 8*3*64] bf16 strip stationaries, block-diagonal per chunk.

    Chunk block c occupies partitions 20c..20c+19 (2 ones rows + 18 data
    rows) and psum rows 16c..16c+15. Column block (f*3+n)*64 holds the
    tap-n stationary for channel f covering all 4 chunks.
    """
    out = np.zeros((80, F_PER_CORE * 3 * 64), dtype=np.float32)
    for f in range(F_PER_CORE):
        b_f = int(b_core[f])
        bh = b_f >> 3
        bl = b_f - 8 * bh
        for n in range(3):
            base = (f * 3 + n) * 64
            for c in range(4):
                col0 = base + 16 * c
                row0 = 20 * c
                if n == 0:
                    out[row0 + 0, col0 : col0 + 16] = float(8 * bh)
                    out[row0 + 1, col0 : col0 + 16] = float(bl)
                for m in range(3):
                    wv = float(int(w_core[f, m, n, 0]))
                    idx = np.arange(16)
                    out[row0 + 2 + idx + m, col0 + idx] = wv
    return out.astype(ml_dtypes.bfloat16)


_PROGRAM_CACHE = {}


def _build_program():
    import concourse.bass as bass
    import concourse.tile as tile
    from concourse import bacc, mybir

    nc = bacc.Bacc(
        "TRN2", target_bir_lowering=False, debug=False, num_devices=N_CORES
    )
    dt = mybir.dt
    Alu = mybir.AluOpType
    Act = mybir.ActivationFunctionType

    xa_d = nc.dram_tensor(
        "x_aug", [F_PER_CORE, 4, 128, FREE_IN], dt.bfloat16, kind="ExternalInput"
    ).ap()
    xs_d = nc.dram_tensor(
        "x_strip", [F_PER_CORE, 80, 520], dt.bfloat16, kind="ExternalInput"
    ).ap()
    lhsT_d = nc.dram_tensor(
        "lhsT", [128, F_PER_CORE * 3 * 124], dt.bfloat16, kind="ExternalInput"
    ).ap()
    lhsT2_d = nc.dram_tensor(
        "lhsT2", [80, F_PER_CORE * 3 * 64], dt.bfloat16, kind="ExternalInput"
    ).ap()
    y_d = nc.dram_tensor(
        "y", [F_PER_CORE, H_OUT, WD_OUT], dt.int8, kind="ExternalOutput"
    ).ap()

    groups = []
    for f in range(F_PER_CORE):
        for wi, (r0, m_r) in enumerate(FULL_WINDOWS):
            groups.append(("full", f, wi, r0, m_r))
        groups.append(("strip", f, 0, STRIP_R0, STRIP_M))

    with tile.TileContext(nc) as tc:
        with (
            tc.tile_pool(name="const", bufs=1) as const_pool,
            tc.tile_pool(name="xin", bufs=3) as x_pool,
            tc.tile_pool(name="psum", bufs=2, space="PSUM") as psum_pool,
            tc.tile_pool(name="af", bufs=3) as af_pool,
            tc.tile_pool(name="out8", bufs=4) as o_pool,
            tc.tile_pool(name="hi", bufs=2) as hi_pool,
            tc.tile_pool(name="lo", bufs=2) as lo_pool,
            tc.tile_pool(name="q", bufs=2) as q_pool,
            tc.tile_pool(name="qt", bufs=2) as qt_pool,
            tc.tile_pool(name="s", bufs=2) as s_pool,
        ):
            lhsT_t = const_pool.tile([128, F_PER_CORE * 3 * 124], dt.bfloat16)
            nc.sync.dma_start(lhsT_t[:], lhsT_d[:])
            lhsT2_t = const_pool.tile([80, F_PER_CORE * 3 * 64], dt.bfloat16)
            nc.sync.dma_start(lhsT2_t[:], lhsT2_d[:])

            pending_y = []
            c_full = 0

            def flush_y(limit):
                while len(pending_y) > limit:
                    emit = pending_y.pop(0)
                    emit()

            for kind, f, wi, r0, m_r in groups:
                if kind == "full":
                    xt = x_pool.tile([128, FREE_IN], dt.bfloat16)
                    nc.sync.dma_start(xt[:], xa_d[f, wi])
                    ps = psum_pool.tile([124, WD_OUT], dt.float32)
                    for n in range(3):
                        base = (f * 3 + n) * 124
                        for c in range(4):
                            nc.tensor.matmul(
                                ps[0:124, c * N_CHUNK : (c + 1) * N_CHUNK],
                                lhsT_t[0:128, base : base + 124],
                                xt[0:128, c * N_CHUNK + 4 * n : c * N_CHUNK + 4 * n + N_CHUNK],
                                start=(n == 0),
                                stop=(n == 2),
                                skip_group_check=True,
                            )
                    rows, fd = 124, WD_OUT
                else:
                    xt = x_pool.tile([128, FREE_IN], dt.bfloat16)
                    nc.sync.dma_start(xt[0:80, 0:520], xs_d[f])
                    ps = psum_pool.tile([124, WD_OUT], dt.float32)
                    for n in range(3):
                        base = (f * 3 + n) * 64
                        nc.tensor.matmul(
                            ps[0:64, 0:N_CHUNK],
                            lhsT2_t[0:80, base : base + 64],
                            xt[0:80, 4 * n : 4 * n + N_CHUNK],
                            start=(n == 0),
                            stop=(n == 2),
                            skip_group_check=True,
                        )
                    rows, fd = 64, N_CHUNK

                af = af_pool.tile([124, WD_OUT], dt.float32)
                nc.scalar.activation(
                    af[0:rows, 0:fd], ps[0:rows, 0:fd], Act.Copy, bias=0.0, scale=1.0
                )

                o8 = o_pool.tile([124, WD_OUT], dt.int8)
                use_dve = kind == "full" and (c_full % 10) < DVE_LANE_FRAC
                if kind == "full":
                    c_full += 1
                if use_dve:
                    hi16 = hi_pool.tile([124, WD_OUT], dt.int16)
                    nc.vector.tensor_scalar(
                        hi16[0:rows, 0:fd], af[0:rows, 0:fd],
                        0.0078125, -8.498046875, Alu.mult, Alu.add,
                    )
                    lo16 = lo_pool.tile([124, WD_OUT], dt.int16)
                    nc.vector.scalar_tensor_tensor(
                        lo16[0:rows, 0:fd], hi16[0:rows, 0:fd], -128.0,
                        af[0:rows, 0:fd], Alu.mult, Alu.add,
                    )
                    q16 = q_pool.tile([124, WD_OUT], dt.int16)
                    nc.vector.tensor_scalar(
                        q16[0:rows, 0:fd], lo16[0:rows, 0:fd],
                        9.7265625, -9960.49609375, Alu.mult, Alu.add,
                    )
                    qt16 = qt_pool.tile([124, WD_OUT], dt.float16)
                    nc.vector.tensor_scalar(
                        qt16[0:rows, 0:fd], q16[0:rows, 0:fd],
                        0.00048828125, -0.136474609375, Alu.mult, Alu.add,
                    )
                    s16 = s_pool.tile([124, WD_OUT], dt.int16)
                    nc.vector.scalar_tensor_tensor(
                        s16[0:rows, 0:fd], hi16[0:rows, 0:fd], 0.60791015625,
                        qt16[0:rows, 0:fd], Alu.mult, Alu.add,
                    )
                    nc.vector.tensor_scalar(
                        o8[0:rows, 0:fd], s16[0:rows, 0:fd], 1.0, None, Alu.mult
                    )
                else:
                    nc.gpsimd.tensor_scalar(
                        o8[0:rows, 0:fd], af[0:rows, 0:fd], QS, QD, Alu.mult, Alu.add
                    )

                def make_emit(kind=kind, f=f, r0=r0, o8=o8):
                    def emit():
                        if kind == "full":
                            nc.scalar.dma_start(y_d[f, r0 : r0 + 124, :], o8[0:124, :])
                        else:
                            for c in range(4):
                                nc.scalar.dma_start(
                                    y_d[f, r0 : r0 + STRIP_M,
                                        c * N_CHUNK : (c + 1) * N_CHUNK],
                                    o8[16 * c : 16 * c + 16, 0:N_CHUNK],
                                )
                    return emit

                pending_y.append(make_emit())
                flush_y(2)

            flush_y(0)

    nc.compile()
    return nc


def _make_in_maps(x: np.ndarray, w: np.ndarray, b: np.ndarray) -> list:
    bf16 = ml_dtypes.bfloat16
    in_maps = []
    for core in range(N_CORES):
        lo = core * F_PER_CORE
        hi = lo + F_PER_CORE
        x_bf = (
            np.ascontiguousarray(x[lo:hi])
            .reshape(F_PER_CORE, H_IN, FREE_IN)
            .astype(bf16)
        )
        x_aug = np.ones((F_PER_CORE, 4, 128, FREE_IN), dtype=bf16)
        for wi, (r0, m_r) in enumerate(FULL_WINDOWS):
            x_aug[:, wi, 2:128, :] = x_bf[:, r0 : r0 + 126, :]
        x_strip = np.ones((F_PER_CORE, 4, STRIP_KB, 520), dtype=bf16)
        for c in range(4):
            x_strip[:, c, 2:STRIP_KB, :] = x_bf[
                :, STRIP_R0 : STRIP_R0 + 18, c * WD_OUT // 4 : c * WD_OUT // 4 + 520
            ]
        in_maps.append(
            {
                "x_aug": x_aug,
                "x_strip": x_strip.reshape(F_PER_CORE, 80, 520),
                "lhsT": _build_lhsT(w[lo:hi], b[lo:hi]),
                "lhsT2": _build_lhsT2(w[lo:hi], b[lo:hi]),
            }
        )
    return in_maps


def kernel(x: np.ndarray, w: np.ndarray, b: np.ndarray) -> np.ndarray:
    """x: int8 [64, 514, 514, 4]; w: int8 [64, 3, 3, 1]; b: int32 [64].

    Returns int8 [64, 512, 512, 4].
    """
    from concourse.bass_utils import run_bass_kernel_spmd

    if "nc" not in _PROGRAM_CACHE:
        _PROGRAM_CACHE["nc"] = _build_program()
    nc = _PROGRAM_CACHE["nc"]

    F = x.shape[0]
    assert F == N_CORES * F_PER_CORE

    in_maps = _make_in_maps(x, w, b)
    res = run_bass_kernel_spmd(nc, in_maps, core_ids=list(range(N_CORES)))

    out = np.empty((F, H_OUT, 512, D), dtype=np.int8)
    for core in range(N_CORES):
        lo = core * F_PER_CORE
        y = res.results[core]["y"]  # [8, 512, 2048] int8
        out[lo : lo + F_PER_CORE] = y.reshape(F_PER_CORE, H_OUT, 512, D)
    return out


# revision 3
# speedup vs baseline: 1.4893x; 1.2147x over previous
"""Trainium2 Bass kernel for nn_Conv2D_80796924772741.

Depthwise (grouped, F=64) 3x3 valid conv over [F, 514, 514, 4] int8 with
per-channel int8 weights + int32 bias, followed by exact fixed-point requant
  res = (acc * 19920 + 2^21) >> 22 ;  out = clip(res - 5, -128, 127) int8
(reduced_mantissa 19920 = 1245 * 16 -> res = (acc*1245 + 2^17) >> 18).

Sharding: F=64 split across 8 NeuronCores (8 channels each).

Per-core pipeline, per (channel, H-window) group ([M<=124 rows, 2048 cols];
last 16 output rows packed 4-chunks-into-partitions as a [64, 512] strip):
 - PE:  conv via Toeplitz-band stationary matmuls over H-windows (3 W-taps,
        H-taps in the band diagonals, bias rides two all-ones rhs rows).
        PSUM = A = acc + b, exact fp32 (|A| <= 146161 < 2^24).
 - ACT: af32 = Copy(PSUM) -> SBUF fp32 (exact).
 - requant, one of two exact lanes (both verified bit-exact on HW for every
   possible A):
   lane G (GpSimd, 1 op, internally > fp32 precision):
        o8 = sat8(RNE(af * 1245/2^18 + (2^-19 - 5)))  == clip(res - 5)
   lane D (DVE, 6 ops, split A = 128*(h'+8) + (lo+1024)):
        hi16 = int16(af*2^-7 - 8.498046875)           = floor(A/128) - 8
        lo16 = int16(hi16*-128 + af)                  = A - 128*hi' (in [1024,1152))
        q16  = int16(lo16*9.7265625 - 9960.49609375)  = floor(lo*1245/128)
        qt16 = fp16(q16*2^-11 - 0.136474609375)       (exact in fp16)
        S16  = int16(hi16*0.60791015625 + qt16)       = res - 5 (RNE, no ties)
        o8   = int8(S16)                              (saturating == clip)
 - DMA y (scalar-engine HWDGE queue, delayed 2 groups to avoid HOL).
x is fed pre-converted to bf16 by the host with the two all-ones bias rows
baked in front of each window, so every x load is a single contiguous DMA.
"""

import numpy as np
import ml_dtypes

F_PER_CORE = 8
H_IN = 514
W_IN = 514
D = 4
H_OUT = 512
WD_OUT = 2048  # 512 * 4
FREE_IN = W_IN * D  # 2056
N_CHUNK = 512
N_CORES = 8

FULL_WINDOWS = [(0, 124), (124, 124), (248, 124), (372, 124)]
STRIP_R0 = 496
STRIP_M = 16  # output rows per chunk block
STRIP_KB = 20  # partitions per chunk block: 2 ones + 16+2 data rows

QS = 0.004749298095703125  # 1245 / 2^18
QD = (2.0 ** -19) - 5.0



def _build_lhsT(w_core: np.ndarray, b_core: np.ndarray) -> np.ndarray:
    """[128, 8*3*124] bf16 stationary: per (channel, w-tap) a Toeplitz band.

    Column block (f*3 + n)*124 : +124 holds T_n for channel f.
    T_n[2 + i + m, i] = w[f, m, n]  (rows 2.. are conv data partitions)
    T_0[0, i] = 8*floor(b/8) ; T_0[1, i] = b mod 8  (bias rows, multiplied
    by all-ones rhs partitions 0/1; both parts bf16-exact).
    """
    out = np.zeros((128, F_PER_CORE * 3 * 124), dtype=np.float32)
    for f in range(F_PER_CORE):
        b_f = int(b_core[f])
        bh = b_f >> 3
        bl = b_f - 8 * bh
        for n in range(3):
            base = (f * 3 + n) * 124
            if n == 0:
                out[0, base : base + 124] = float(8 * bh)
                out[1, base : base + 124] = float(bl)
            for m in range(3):
                wv = float(int(w_core[f, m, n, 0]))
                idx = np.arange(124)
                out[2 + idx + m, base + idx] = wv
    return out.astype(ml_dtypes.bfloat16)


def _build_lhsT2(w_core: np.ndarray, b_core: np.ndarray) -> np.ndarray:
    """[80, 8*3*64] bf16 strip stationaries, block-diagonal per chunk.

    Chunk block c occupies partitions 20c..20c+19 (2 ones rows + 18 data
    rows) and psum rows 16c..16c+15. Column block (f*3+n)*64 holds the
    tap-n stationary for channel f covering all 4 chunks.
    """
    out = np.zeros((80, F_PER_CORE * 3 * 64), dtype=np.float32)
    for f in range(F_PER_CORE):
        b_f = int(b_core[f])
        bh = b_f >> 3
        bl = b_f - 8 * bh
        for n in range(3):
            base = (f * 3 + n) * 64
            for c in range(4):
                col0 = base + 16 * c
                row0 = 20 * c
                if n == 0:
                    out[row0 + 0, col0 : col0 + 16] = float(8 * bh)
                    out[row0 + 1, col0 : col0 + 16] = float(bl)
                for m in range(3):
                    wv = float(int(w_core[f, m, n, 0]))
                    idx = np.arange(16)
                    out[row0 + 2 + idx + m, col0 + idx] = wv
    return out.astype(ml_dtypes.bfloat16)


_PROGRAM_CACHE = {}


def _build_program():
    import concourse.bass as bass
    import concourse.tile as tile
    from concourse import bacc, mybir

    nc = bacc.Bacc(
        "TRN2", target_bir_lowering=False, debug=False, num_devices=N_CORES
    )
    dt = mybir.dt
    Alu = mybir.AluOpType
    Act = mybir.ActivationFunctionType

    xa_d = nc.dram_tensor(
        "x_aug", [F_PER_CORE, 4, 128, FREE_IN], dt.bfloat16, kind="ExternalInput"
    ).ap()
    xs_d = nc.dram_tensor(
        "x_strip", [F_PER_CORE, 80, 520], dt.bfloat16, kind="ExternalInput"
    ).ap()
    lhsT_d = nc.dram_tensor(
        "lhsT", [128, F_PER_CORE * 3 * 124], dt.bfloat16, kind="ExternalInput"
    ).ap()
    lhsT2_d = nc.dram_tensor(
        "lhsT2", [80, F_PER_CORE * 3 * 64], dt.bfloat16, kind="ExternalInput"
    ).ap()
    y_d = nc.dram_tensor(
        "y", [F_PER_CORE, H_OUT, WD_OUT], dt.int8, kind="ExternalOutput"
    ).ap()

    groups = []
    for f in range(F_PER_CORE):
        for wi, (r0, m_r) in enumerate(FULL_WINDOWS):
            groups.append(("full", f, wi, r0, m_r))
        groups.append(("strip", f, 0, STRIP_R0, STRIP_M))

    with tile.TileContext(nc) as tc:
        with (
            tc.tile_pool(name="const", bufs=1) as const_pool,
            tc.tile_pool(name="xin", bufs=3) as x_pool,
            tc.tile_pool(name="psum", bufs=2, space="PSUM") as psum_pool,
            tc.tile_pool(name="af", bufs=3) as af_pool,
            tc.tile_pool(name="out8", bufs=4) as o_pool,
        ):
            lhsT_t = const_pool.tile([128, F_PER_CORE * 3 * 124], dt.bfloat16)
            nc.sync.dma_start(lhsT_t[:], lhsT_d[:])
            lhsT2_t = const_pool.tile([80, F_PER_CORE * 3 * 64], dt.bfloat16)
            nc.sync.dma_start(lhsT2_t[:], lhsT2_d[:])
            warm = const_pool.tile([128, 8], dt.int32)
            nc.gpsimd.memset(warm[:], 0)
            nc.gpsimd.tensor_scalar(warm[:], warm[:], 1, 0, Alu.mult, Alu.add)

            pending_y = []
            c_full = 0

            def flush_y(limit):
                while len(pending_y) > limit:
                    emit = pending_y.pop(0)
                    emit()

            for kind, f, wi, r0, m_r in groups:
                if kind == "full":
                    xt = x_pool.tile([128, FREE_IN], dt.bfloat16)
                    nc.sync.dma_start(xt[:], xa_d[f, wi])
                    ps = psum_pool.tile([124, WD_OUT], dt.float32)
                    for n in range(3):
                        base = (f * 3 + n) * 124
                        for c in range(4):
                            nc.tensor.matmul(
                                ps[0:124, c * N_CHUNK : (c + 1) * N_CHUNK],
                                lhsT_t[0:128, base : base + 124],
                                xt[0:128, c * N_CHUNK + 4 * n : c * N_CHUNK + 4 * n + N_CHUNK],
                                start=(n == 0),
                                stop=(n == 2),
                                skip_group_check=True,
                            )
                    rows, fd = 124, WD_OUT
                else:
                    xt = x_pool.tile([128, FREE_IN], dt.bfloat16)
                    nc.sync.dma_start(xt[0:80, 0:520], xs_d[f])
                    ps = psum_pool.tile([124, WD_OUT], dt.float32)
                    for n in range(3):
                        base = (f * 3 + n) * 64
                        nc.tensor.matmul(
                            ps[0:64, 0:N_CHUNK],
                            lhsT2_t[0:80, base : base + 64],
                            xt[0:80, 4 * n : 4 * n + N_CHUNK],
                            start=(n == 0),
                            stop=(n == 2),
                            skip_group_check=True,
                        )
                    rows, fd = 64, N_CHUNK

                af = af_pool.tile([124, WD_OUT], dt.float32)
                use_dve_copy = kind == "full" and (c_full % 2 == 1)
                if kind == "full":
                    c_full += 1
                if use_dve_copy:
                    nc.vector.tensor_copy(af[0:rows, 0:fd], ps[0:rows, 0:fd])
                else:
                    nc.scalar.activation(
                        af[0:rows, 0:fd], ps[0:rows, 0:fd], Act.Copy,
                        bias=0.0, scale=1.0,
                    )

                o8 = o_pool.tile([124, WD_OUT], dt.int8)
                nc.gpsimd.tensor_scalar(
                    o8[0:rows, 0:fd], af[0:rows, 0:fd], QS, QD, Alu.mult, Alu.add
                )

                def make_emit(kind=kind, f=f, r0=r0, o8=o8):
                    def emit():
                        if kind == "full":
                            nc.scalar.dma_start(y_d[f, r0 : r0 + 124, :], o8[0:124, :])
                        else:
                            for c in range(4):
                                nc.sync.dma_start(
                                    y_d[f, r0 : r0 + STRIP_M,
                                        c * N_CHUNK : (c + 1) * N_CHUNK],
                                    o8[16 * c : 16 * c + 16, 0:N_CHUNK],
                                )
                    return emit

                pending_y.append(make_emit())
                flush_y(2)

            flush_y(0)

    nc.compile()
    return nc


def _make_in_maps(x: np.ndarray, w: np.ndarray, b: np.ndarray) -> list:
    bf16 = ml_dtypes.bfloat16
    in_maps = []
    for core in range(N_CORES):
        lo = core * F_PER_CORE
        hi = lo + F_PER_CORE
        x_bf = (
            np.ascontiguousarray(x[lo:hi])
            .reshape(F_PER_CORE, H_IN, FREE_IN)
            .astype(bf16)
        )
        x_aug = np.ones((F_PER_CORE, 4, 128, FREE_IN), dtype=bf16)
        for wi, (r0, m_r) in enumerate(FULL_WINDOWS):
            x_aug[:, wi, 2:128, :] = x_bf[:, r0 : r0 + 126, :]
        x_strip = np.ones((F_PER_CORE, 4, STRIP_KB, 520), dtype=bf16)
        for c in range(4):
            x_strip[:, c, 2:STRIP_KB, :] = x_bf[
                :, STRIP_R0 : STRIP_R0 + 18, c * WD_OUT // 4 : c * WD_OUT // 4 + 520
            ]
        in_maps.append(
            {
                "x_aug": x_aug,
                "x_strip": x_strip.reshape(F_PER_CORE, 80, 520),
                "lhsT": _build_lhsT(w[lo:hi], b[lo:hi]),
                "lhsT2": _build_lhsT2(w[lo:hi], b[lo:hi]),
            }
        )
    return in_maps


def kernel(x: np.ndarray, w: np.ndarray, b: np.ndarray) -> np.ndarray:
    """x: int8 [64, 514, 514, 4]; w: int8 [64, 3, 3, 1]; b: int32 [64].

    Returns int8 [64, 512, 512, 4].
    """
    from concourse.bass_utils import run_bass_kernel_spmd

    if "nc" not in _PROGRAM_CACHE:
        _PROGRAM_CACHE["nc"] = _build_program()
    nc = _PROGRAM_CACHE["nc"]

    F = x.shape[0]
    assert F == N_CORES * F_PER_CORE

    in_maps = _make_in_maps(x, w, b)
    res = run_bass_kernel_spmd(nc, in_maps, core_ids=list(range(N_CORES)))

    out = np.empty((F, H_OUT, 512, D), dtype=np.int8)
    for core in range(N_CORES):
        lo = core * F_PER_CORE
        y = res.results[core]["y"]  # [8, 512, 2048] int8
        out[lo : lo + F_PER_CORE] = y.reshape(F_PER_CORE, H_OUT, 512, D)
    return out


# revision 4
# speedup vs baseline: 1.5782x; 1.0597x over previous
"""Trainium2 Bass kernel for nn_Conv2D_80796924772741.

Depthwise (grouped, F=64) 3x3 valid conv over [F, 514, 514, 4] int8 with
per-channel int8 weights + int32 bias, followed by exact fixed-point requant
  res = (acc * 19920 + 2^21) >> 22 ;  out = clip(res - 5, -128, 127) int8
(reduced_mantissa 19920 = 1245 * 16 -> res = (acc*1245 + 2^17) >> 18).

Sharding: F=64 split across 8 NeuronCores (8 channels each).

Per-core pipeline, per (channel, H-window) group ([M<=124 rows, 2048 cols];
last 16 output rows packed 4-chunks-into-partitions as a [64, 512] strip):
 - PE:  conv via Toeplitz-band stationary matmuls over H-windows (3 W-taps,
        H-taps in the band diagonals, bias rides two all-ones rhs rows).
        PSUM = A = acc + b, exact fp32 (|A| <= 146161 < 2^24).
 - ACT: af32 = Copy(PSUM) -> SBUF fp32 (exact).
 - requant, one of two exact lanes (both verified bit-exact on HW for every
   possible A):
   lane G (GpSimd, 1 op, internally > fp32 precision):
        o8 = sat8(RNE(af * 1245/2^18 + (2^-19 - 5)))  == clip(res - 5)
   lane D (DVE, 6 ops, split A = 128*(h'+8) + (lo+1024)):
        hi16 = int16(af*2^-7 - 8.498046875)           = floor(A/128) - 8
        lo16 = int16(hi16*-128 + af)                  = A - 128*hi' (in [1024,1152))
        q16  = int16(lo16*9.7265625 - 9960.49609375)  = floor(lo*1245/128)
        qt16 = fp16(q16*2^-11 - 0.136474609375)       (exact in fp16)
        S16  = int16(hi16*0.60791015625 + qt16)       = res - 5 (RNE, no ties)
        o8   = int8(S16)                              (saturating == clip)
 - DMA y (scalar-engine HWDGE queue, delayed 2 groups to avoid HOL).
x is fed pre-converted to bf16 by the host with the two all-ones bias rows
baked in front of each window, so every x load is a single contiguous DMA.
"""

import numpy as np
import ml_dtypes

F_PER_CORE = 8
H_IN = 514
W_IN = 514
D = 4
H_OUT = 512
WD_OUT = 2048  # 512 * 4
FREE_IN = W_IN * D  # 2056
N_CHUNK = 512
N_CORES = 8

FULL_WINDOWS = [(0, 124), (124, 124), (248, 124), (372, 124)]
STRIP_R0 = 496
STRIP_M = 16  # output rows per chunk block
STRIP_KB = 20  # partitions per chunk block: 2 ones + 16+2 data rows

QS = 0.004749298095703125  # 1245 / 2^18
QD = (2.0 ** -19) - 5.0



def _build_lhsT(w_core: np.ndarray, b_core: np.ndarray) -> np.ndarray:
    """[128, 8*3*124] bf16 stationary: per (channel, w-tap) a Toeplitz band.

    Column block (f*3 + n)*124 : +124 holds T_n for channel f.
    T_n[2 + i + m, i] = w[f, m, n]  (rows 2.. are conv data partitions)
    T_0[0, i] = 8*floor(b/8) ; T_0[1, i] = b mod 8  (bias rows, multiplied
    by all-ones rhs partitions 0/1; both parts bf16-exact).
    """
    out = np.zeros((128, F_PER_CORE * 3 * 124), dtype=np.float32)
    for f in range(F_PER_CORE):
        b_f = int(b_core[f])
        bh = b_f >> 3
        bl = b_f - 8 * bh
        for n in range(3):
            base = (f * 3 + n) * 124
            if n == 0:
                out[0, base : base + 124] = float(8 * bh)
                out[1, base : base + 124] = float(bl)
            for m in range(3):
                wv = float(int(w_core[f, m, n, 0]))
                idx = np.arange(124)
                out[2 + idx + m, base + idx] = wv
    return out.astype(ml_dtypes.bfloat16)


def _build_lhsT2(w_core: np.ndarray, b_core: np.ndarray) -> np.ndarray:
    """[80, 8*3*64] bf16 strip stationaries, block-diagonal per chunk.

    Chunk block c occupies partitions 20c..20c+19 (2 ones rows + 18 data
    rows) and psum rows 16c..16c+15. Column block (f*3+n)*64 holds the
    tap-n stationary for channel f covering all 4 chunks.
    """
    out = np.zeros((80, F_PER_CORE * 3 * 64), dtype=np.float32)
    for f in range(F_PER_CORE):
        b_f = int(b_core[f])
        bh = b_f >> 3
        bl = b_f - 8 * bh
        for n in range(3):
            base = (f * 3 + n) * 64
            for c in range(4):
                col0 = base + 16 * c
                row0 = 20 * c
                if n == 0:
                    out[row0 + 0, col0 : col0 + 16] = float(8 * bh)
                    out[row0 + 1, col0 : col0 + 16] = float(bl)
                for m in range(3):
                    wv = float(int(w_core[f, m, n, 0]))
                    idx = np.arange(16)
                    out[row0 + 2 + idx + m, col0 + idx] = wv
    return out.astype(ml_dtypes.bfloat16)


_PROGRAM_CACHE = {}


def _build_program():
    import concourse.bass as bass
    import concourse.tile as tile
    from concourse import bacc, mybir

    nc = bacc.Bacc(
        "TRN2", target_bir_lowering=False, debug=False, num_devices=N_CORES
    )
    dt = mybir.dt
    Alu = mybir.AluOpType
    Act = mybir.ActivationFunctionType

    xa_d = nc.dram_tensor(
        "x_aug", [F_PER_CORE, 4, 128, FREE_IN], dt.bfloat16, kind="ExternalInput"
    ).ap()
    xs_d = nc.dram_tensor(
        "x_strip", [F_PER_CORE, 80, 520], dt.bfloat16, kind="ExternalInput"
    ).ap()
    lhsT_d = nc.dram_tensor(
        "lhsT", [128, F_PER_CORE * 3 * 124], dt.bfloat16, kind="ExternalInput"
    ).ap()
    lhsT2_d = nc.dram_tensor(
        "lhsT2", [80, F_PER_CORE * 3 * 64], dt.bfloat16, kind="ExternalInput"
    ).ap()
    y_d = nc.dram_tensor(
        "y", [F_PER_CORE, H_OUT, WD_OUT], dt.int8, kind="ExternalOutput"
    ).ap()

    groups = []
    for f in range(F_PER_CORE):
        for wi, (r0, m_r) in enumerate(FULL_WINDOWS):
            groups.append(("full", f, wi, r0, m_r))
        groups.append(("strip", f, 0, STRIP_R0, STRIP_M))

    with tile.TileContext(nc) as tc:
        with (
            tc.tile_pool(name="const", bufs=1) as const_pool,
            tc.tile_pool(name="xin", bufs=4) as x_pool,
            tc.tile_pool(name="psum", bufs=2, space="PSUM") as psum_pool,
            tc.tile_pool(name="af2", bufs=3) as af2_pool,
            tc.tile_pool(name="o82", bufs=3) as o82_pool,
            tc.tile_pool(name="afs", bufs=2) as afs_pool,
            tc.tile_pool(name="o8s", bufs=2) as o8s_pool,
        ):
            lhsT_t = const_pool.tile([128, F_PER_CORE * 3 * 124], dt.bfloat16)
            nc.sync.dma_start(lhsT_t[:], lhsT_d[:])
            lhsT2_t = const_pool.tile([80, F_PER_CORE * 3 * 64], dt.bfloat16)
            nc.sync.dma_start(lhsT2_t[:], lhsT2_d[:])
            warm = const_pool.tile([128, 8], dt.int32)
            nc.gpsimd.memset(warm[:], 0)
            nc.gpsimd.tensor_scalar(warm[:], warm[:], 1, 0, Alu.mult, Alu.add)

            pending_y = []
            c_full = 0

            def flush_y(limit):
                while len(pending_y) > limit:
                    emit = pending_y.pop(0)
                    emit()

            af2 = None
            for kind, f, wi, r0, m_r in groups:
                if kind == "full":
                    xt = x_pool.tile([128, FREE_IN], dt.bfloat16)
                    nc.sync.dma_start(xt[:], xa_d[f, wi])
                    ps = psum_pool.tile([124, WD_OUT], dt.float32)
                    for n in range(3):
                        base = (f * 3 + n) * 124
                        for c in range(4):
                            nc.tensor.matmul(
                                ps[0:124, c * N_CHUNK : (c + 1) * N_CHUNK],
                                lhsT_t[0:128, base : base + 124],
                                xt[0:128, c * N_CHUNK + 4 * n : c * N_CHUNK + 4 * n + N_CHUNK],
                                start=(n == 0),
                                stop=(n == 2),
                                skip_group_check=True,
                            )
                    half = wi % 2
                    if half == 0:
                        af2 = af2_pool.tile([124, 2 * WD_OUT], dt.float32)
                    dst = af2[0:124, half * WD_OUT : (half + 1) * WD_OUT]
                    if c_full % 2 == 1:
                        nc.vector.tensor_copy(dst, ps[0:124, :])
                    else:
                        nc.scalar.activation(
                            dst, ps[0:124, :], Act.Copy, bias=0.0, scale=1.0
                        )
                    c_full += 1
                    if half == 1:
                        o82 = o82_pool.tile([124, 2 * WD_OUT], dt.int8)
                        nc.gpsimd.tensor_scalar(
                            o82[:], af2[:], QS, QD, Alu.mult, Alu.add
                        )
                        for h, rr in ((0, r0 - 124), (1, r0)):
                            def emit_full(f=f, rr=rr, o82=o82, h=h):
                                nc.scalar.dma_start(
                                    y_d[f, rr : rr + 124, :],
                                    o82[0:124, h * WD_OUT : (h + 1) * WD_OUT],
                                )
                            pending_y.append(emit_full)
                else:
                    xt = x_pool.tile([128, FREE_IN], dt.bfloat16)
                    nc.sync.dma_start(xt[0:80, 0:520], xs_d[f])
                    ps = psum_pool.tile([124, WD_OUT], dt.float32)
                    for n in range(3):
                        base = (f * 3 + n) * 64
                        nc.tensor.matmul(
                            ps[0:64, 0:N_CHUNK],
                            lhsT2_t[0:80, base : base + 64],
                            xt[0:80, 4 * n : 4 * n + N_CHUNK],
                            start=(n == 0),
                            stop=(n == 2),
                            skip_group_check=True,
                        )
                    afs = afs_pool.tile([124, N_CHUNK], dt.float32)
                    nc.scalar.activation(
                        afs[0:64, :], ps[0:64, 0:N_CHUNK], Act.Copy,
                        bias=0.0, scale=1.0,
                    )
                    o8s = o8s_pool.tile([124, N_CHUNK], dt.int8)
                    nc.gpsimd.tensor_scalar(
                        o8s[0:64, :], afs[0:64, :], QS, QD, Alu.mult, Alu.add
                    )
                    def emit_strip(f=f, r0=r0, o8s=o8s):
                        for c in range(4):
                            nc.scalar.dma_start(
                                y_d[f, r0 : r0 + STRIP_M,
                                    c * N_CHUNK : (c + 1) * N_CHUNK],
                                o8s[16 * c : 16 * c + 16, 0:N_CHUNK],
                            )
                    pending_y.append(emit_strip)
                flush_y(3)

            flush_y(0)

    nc.compile()
    return nc


def _make_in_maps(x: np.ndarray, w: np.ndarray, b: np.ndarray) -> list:
    bf16 = ml_dtypes.bfloat16
    in_maps = []
    for core in range(N_CORES):
        lo = core * F_PER_CORE
        hi = lo + F_PER_CORE
        x_bf = (
            np.ascontiguousarray(x[lo:hi])
            .reshape(F_PER_CORE, H_IN, FREE_IN)
            .astype(bf16)
        )
        x_aug = np.ones((F_PER_CORE, 4, 128, FREE_IN), dtype=bf16)
        for wi, (r0, m_r) in enumerate(FULL_WINDOWS):
            x_aug[:, wi, 2:128, :] = x_bf[:, r0 : r0 + 126, :]
        x_strip = np.ones((F_PER_CORE, 4, STRIP_KB, 520), dtype=bf16)
        for c in range(4):
            x_strip[:, c, 2:STRIP_KB, :] = x_bf[
                :, STRIP_R0 : STRIP_R0 + 18, c * WD_OUT // 4 : c * WD_OUT // 4 + 520
            ]
        in_maps.append(
            {
                "x_aug": x_aug,
                "x_strip": x_strip.reshape(F_PER_CORE, 80, 520),
                "lhsT": _build_lhsT(w[lo:hi], b[lo:hi]),
                "lhsT2": _build_lhsT2(w[lo:hi], b[lo:hi]),
            }
        )
    return in_maps


def kernel(x: np.ndarray, w: np.ndarray, b: np.ndarray) -> np.ndarray:
    """x: int8 [64, 514, 514, 4]; w: int8 [64, 3, 3, 1]; b: int32 [64].

    Returns int8 [64, 512, 512, 4].
    """
    from concourse.bass_utils import run_bass_kernel_spmd

    if "nc" not in _PROGRAM_CACHE:
        _PROGRAM_CACHE["nc"] = _build_program()
    nc = _PROGRAM_CACHE["nc"]

    F = x.shape[0]
    assert F == N_CORES * F_PER_CORE

    in_maps = _make_in_maps(x, w, b)
    res = run_bass_kernel_spmd(nc, in_maps, core_ids=list(range(N_CORES)))

    out = np.empty((F, H_OUT, 512, D), dtype=np.int8)
    for core in range(N_CORES):
        lo = core * F_PER_CORE
        y = res.results[core]["y"]  # [8, 512, 2048] int8
        out[lo : lo + F_PER_CORE] = y.reshape(F_PER_CORE, H_OUT, 512, D)
    return out
